# revision 22
# baseline (speedup 1.0000x reference)
"""Gated multi-head attention (AlphaFold-style) on 8 TRN2 NeuronCores.

Sharding: data-parallel over batch B=32 -> 4 batches per core; zero collectives.

Layout strategy ("transposed land"): all on-device tensors keep the softmax
key axis (k) on SBUF partitions so the big bias tensors stream in naturally
after a host-side transpose, exp() fuses the per-key row bias via the ACT
bias port, and the softmax denominator comes out of the PE via a ones[128,32]
stationary matmul (which also pre-broadcasts 1/sum across each head's 32
partition rows for free). Host-side work is layout-only (transpose/reshape);
all arithmetic runs on device, bf16 matmuls with fp32 PSUM accumulation.

  qhT[hc, q]  = (query_w*scale)[a,hc]^T @ q_dataT[a,q]           (PE)
  khT[hc, k]  =  key_w^T @ m_dataT                               (PE)
  vb[k, hc]   =  (m_dataT^T-chunks @ value_w) -> bf16            (PE + DVE)
  gateT[hc,q] =  sigmoid(gating_w^T @ q_dataT + gating_b)        (PE + ACT)
  logitsT(h)[k,q] = khT_h^T-slices @ qhT_h  (row-tiled 4 heads)  (PE)
  psum += Id @ (nbT + bbT)                  (PE id-add; DVE pre-add)
  PT(h)[k,q]  = exp(psum + bias_row[k])  -> bf16                 (ACT)
  avT, sums   = col-tiled matmuls over k; AV lags one group
                behind exp in the PE stream to stay warm         (PE, bf16)
  wag         = avT * gateT * approx(1/sums)                     (DVE)
  outT[o, q]  = output_w^T-chunks @ wag + output_b               (PE + DVE)

Output is produced as [o, q] per batch and un-transposed on the host.
"""

import numpy as np

import concourse.bass as bass
import concourse.mybir as mybir
from concourse import bacc
from concourse.tile import TileContext
from concourse.masks import make_identity
from concourse.bass_utils import run_bass_kernel_spmd

B, Q, K, A, H, C, O = 32, 512, 512, 256, 8, 32, 256
CORES = 8
BLOC = B // CORES          # batches per core
NKC = K // 128             # k chunks
F32 = mybir.dt.float32
BF16 = mybir.dt.bfloat16
KEY_SCALE = float(C) ** -0.5
AF = mybir.ActivationFunctionType


def build_nc():
    nc = bacc.Bacc(None, target_bir_lowering=False)

    # --- DRAM parameters (per-core shards; names match in_maps keys) ---
    p_qT = nc.declare_dram_parameter("qT", [BLOC, A, Q], BF16, isOutput=False)
    p_mT = nc.declare_dram_parameter("mT", [BLOC, A, K], BF16, isOutput=False)
    p_br = nc.declare_dram_parameter("biasr", [BLOC, K], F32, isOutput=False)
    p_bbT = nc.declare_dram_parameter("bbT", [BLOC, H, K, Q], BF16, isOutput=False)
    p_nbT = nc.declare_dram_parameter("nbT", [H, K, Q], BF16, isOutput=False)
    p_qw = nc.declare_dram_parameter("qw", [A, H * C], F32, isOutput=False)
    p_kw = nc.declare_dram_parameter("kw", [A, H * C], F32, isOutput=False)
    p_vw = nc.declare_dram_parameter("vw", [A, H * C], F32, isOutput=False)
    p_gw = nc.declare_dram_parameter("gw", [A, H * C], F32, isOutput=False)
    p_gb = nc.declare_dram_parameter("gb", [H * C], F32, isOutput=False)
    p_ow = nc.declare_dram_parameter("ow", [H * C, O], F32, isOutput=False)
    p_ob = nc.declare_dram_parameter("ob", [O], F32, isOutput=False)
    p_out = nc.declare_dram_parameter("out", [BLOC, O, Q], F32, isOutput=True)

    with TileContext(nc) as tc:
        with (
            tc.tile_pool(name="const", bufs=1) as const,
            tc.tile_pool(name="nbres", bufs=1) as nbres,
            tc.tile_pool(name="data", bufs=2) as data,
            tc.tile_pool(name="proj", bufs=2) as proj,
            tc.tile_pool(name="bbt", bufs=12) as bbtp,
            tc.tile_pool(name="nbb", bufs=12) as nbbp,
            tc.tile_pool(name="pt", bufs=8) as ptp,
            tc.tile_pool(name="post", bufs=2) as post,
            tc.tile_pool(name="ps", bufs=2, space="PSUM") as psp,
            tc.tile_pool(name="avps", bufs=2, space="PSUM") as avps,
            tc.tile_pool(name="sumps", bufs=2, space="PSUM") as sumps,
        ):
            # ---------- one-time constants ----------
            ident = const.tile([128, 128], BF16)
            make_identity(nc, ident)
            ones = const.tile([128, 32], BF16)
            nc.vector.memset(ones, 1.0)

            # weights: [a, hc] -> [128, ka, hc], cast to bf16 on device;
            # key_scale folded into qw here.
            qw_sb = const.tile([128, 2, 256], BF16)
            kw_sb = const.tile([128, 2, 256], BF16)
            vw_sb = const.tile([128, 2, 256], BF16)
            gw_sb = const.tile([128, 2, 256], BF16)
            ow_sb = const.tile([128, 2, 256], BF16)
            for t, p, pat, scl in (
                (qw_sb, p_qw, "(ka p) hc -> p ka hc", KEY_SCALE),
                (kw_sb, p_kw, "(ka p) hc -> p ka hc", None),
                (vw_sb, p_vw, "(ka p) hc -> p ka hc", None),
                (gw_sb, p_gw, "(ka p) hc -> p ka hc", None),
                (ow_sb, p_ow, "(kh p) o -> p kh o", None),
            ):
                wstage = data.tile([128, 2, 256], F32, tag="stage")
                nc.sync.dma_start(out=wstage, in_=p.rearrange(pat, p=128))
                if scl is None:
                    nc.vector.tensor_copy(out=t, in_=wstage)
                else:
                    nc.vector.tensor_scalar_mul(out=t, in0=wstage, scalar1=scl)
            gb_sb = const.tile([128, 2], F32)
            nc.sync.dma_start(out=gb_sb, in_=p_gb.rearrange("(m p) -> p m", p=128))
            ob_sb = const.tile([128, 2], F32)
            nc.sync.dma_start(out=ob_sb, in_=p_ob.rearrange("(m p) -> p m", p=128))

            # nonbatched bias resident as bf16: [128, h, kc, q]
            # (loaded lazily, interleaved into batch 0's attention pipeline)
            nbt16 = nbres.tile([128, H, NKC, Q], BF16)

            # ---------- per-batch pipeline ----------
            # post(b-1) is emitted after proj(b) so the PE can run batch b's
            # projections while the DVE finishes b-1's normalize chain.
            def make_post(b, avt, smt, gate):
                def post_fn():
                    recb = post.tile([128, 2, Q], F32, tag="recb")
                    for t in range(2):
                        nc.vector.reciprocal_approx_fast(
                            out=recb[:, t], in_=smt[t])
                    grec = post.tile([128, 2, Q], F32, tag="grec")
                    wag = post.tile([128, 2, Q], BF16, tag="wag")
                    for t in range(2):
                        nc.vector.tensor_mul(
                            out=grec[:, t], in0=gate[:, t], in1=recb[:, t])
                        nc.vector.tensor_mul(
                            out=wag[:, t], in0=avt[t], in1=grec[:, t])
                    outT = post.tile([128, 2, Q], F32, tag="outT")
                    po2 = psp.tile([128, 2, Q], F32, tag="mm")
                    for mo in range(2):
                        oslc = slice(mo * 128, (mo + 1) * 128)
                        for kh in range(2):
                            nc.tensor.matmul(
                                po2[:, mo], ow_sb[:, kh, oslc], wag[:, kh],
                                start=(kh == 0), stop=(kh == 1))
                    for mo in range(2):
                        nc.vector.tensor_scalar_add(
                            out=outT[:, mo], in0=po2[:, mo],
                            scalar1=ob_sb[:, mo:mo + 1])
                    nc.gpsimd.dma_start(
                        out=p_out[b].rearrange("(mo p) q -> p mo q", p=128),
                        in_=outT)
                return post_fn

            pending_post = None
            for b in range(BLOC):
                qT_sb = data.tile([128, 2, Q], BF16, tag="qT")
                nc.sync.dma_start(
                    out=qT_sb, in_=p_qT[b].rearrange("(ka p) q -> p ka q", p=128)
                )
                mT_sb = data.tile([128, 2, K], BF16, tag="mT")
                nc.sync.dma_start(
                    out=mT_sb, in_=p_mT[b].rearrange("(ka p) q -> p ka q", p=128)
                )
                br_sb = data.tile([128, NKC], F32, tag="br")
                nc.sync.dma_start(
                    out=br_sb, in_=p_br[b].rearrange("(kc p) -> p kc", p=128)
                )

                # --- projections ---
                qhT = proj.tile([128, 2, Q], BF16, tag="qhT")
                khT = proj.tile([128, 2, K], BF16, tag="khT")
                gate = proj.tile([128, 2, Q], F32, tag="gate")
                for m in range(2):
                    mslc = slice(m * 128, (m + 1) * 128)
                    pqk = psp.tile([128, 2, Q], F32, tag="mm")
                    pgv = psp.tile([128, 2, Q], F32, tag="mm")
                    pq, pk, pg = pqk[:, 0], pqk[:, 1], pgv[:, 0]
                    for ka in range(2):
                        st, sp = ka == 0, ka == 1
                        nc.tensor.matmul(
                            pq, qw_sb[:, ka, mslc], qT_sb[:, ka], start=st, stop=sp)
                        nc.tensor.matmul(
                            pk, kw_sb[:, ka, mslc], mT_sb[:, ka], start=st, stop=sp)
                        nc.tensor.matmul(
                            pg, gw_sb[:, ka, mslc], qT_sb[:, ka], start=st, stop=sp)
                    nc.vector.tensor_copy(out=qhT[:, m], in_=pq)
                    nc.vector.tensor_copy(out=khT[:, m], in_=pk)
                    nc.scalar.activation(gate[:, m], pg, AF.Sigmoid,
                                         bias=gb_sb[:, m:m + 1], scale=1.0)

                # v projection -> bf16 [k(part), kc, hc]
                vb = proj.tile([128, NKC, 256], BF16, tag="vb")
                for kch in range(2):
                    pv2 = psp.tile([128, 2, Q], F32, tag="mm")
                    for kci in range(2):
                        kc = 2 * kch + kci
                        kslc = slice(kc * 128, (kc + 1) * 128)
                        pv = pv2[:, kci, 0:256]
                        for ka in range(2):
                            nc.tensor.matmul(
                                pv, mT_sb[:, ka, kslc], vw_sb[:, ka],
                                start=(ka == 0), stop=(ka == 1))
                        nc.vector.tensor_copy(out=vb[:, kc], in_=pv)

                # previous batch's normalize/output tail
                if pending_post is not None:
                    pending_post()
                    pending_post = None

                # --- attention core ---
                av0 = avps.tile([128, Q], F32, tag="av")     # heads 0-3
                av1 = avps.tile([128, Q], F32, tag="av")     # heads 4-7
                sm0 = sumps.tile([128, Q], F32, tag="sm")    # per-head sums x32
                sm1 = sumps.tile([128, Q], F32, tag="sm")
                avt = (av0, av1)
                smt = (sm0, sm1)

                def emit_av(g):
                    g_heads, g_pts, g_kc = g
                    for i2, h2 in enumerate(g_heads):
                        j2 = h2 % 4
                        nc.tensor.matmul(
                            avt[h2 // 4][32 * j2:32 * j2 + 32],
                            vb[:, g_kc, 32 * h2:32 * h2 + 32],
                            g_pts[i2],
                            start=(g_kc == 0), stop=(g_kc == NKC - 1),
                            tile_position=(0, 32 * j2), skip_group_check=True)
                    for i2, h2 in enumerate(g_heads):
                        j2 = h2 % 4
                        nc.tensor.matmul(
                            smt[h2 // 4][32 * j2:32 * j2 + 32],
                            ones, g_pts[i2],
                            start=(g_kc == 0), stop=(g_kc == NKC - 1),
                            tile_position=(0, 32 * j2), skip_group_check=True)

                pending = None
                for kc in range(NKC):
                    kslc = slice(kc * 128, (kc + 1) * 128)
                    for sg in range(4):       # subgroup: heads 2*sg, 2*sg+1
                        hs = sg // 2
                        heads = [2 * sg, 2 * sg + 1]
                        if b == 0 and kc == 0:
                            for h in heads:
                                nc.sync.dma_start(
                                    out=nbt16[:, h],
                                    in_=p_nbT[h].rearrange(
                                        "(kc2 p) q -> p kc2 q", p=128),
                                )
                        # pre-add biases on DVE (one op per head)
                        nbbs = []
                        for i, h in enumerate(heads):
                            bbt = bbtp.tile([128, Q], BF16, tag="bbt")
                            nc.sync.dma_start(out=bbt, in_=p_bbT[b, h, kslc])
                            nbb = nbbp.tile([128, Q], BF16, tag="nbb")
                            nc.vector.tensor_add(
                                out=nbb, in0=nbt16[:, h, kc], in1=bbt)
                            nbbs.append(nbb)
                        # row-tiled QK^T (2 heads concurrent, one 2-bank tile)
                        qk2 = psp.tile([128, 2, Q], F32, tag="mm")
                        for i, h in enumerate(heads):
                            j = h % 4
                            jslc = slice(32 * j, 32 * j + 32)
                            nc.tensor.matmul(
                                qk2[:, i],
                                khT[jslc, h // 4, kslc],
                                qhT[jslc, h // 4],
                                start=True, stop=False,
                                tile_position=(32 * j, 0))
                        # identity-add of biases into psum
                        for i, h in enumerate(heads):
                            nc.tensor.matmul(
                                qk2[:, i], ident, nbbs[i],
                                start=False, stop=True)
                        # AV/sums of the PREVIOUS subgroup fill the PE while
                        # this one's exp runs (warm PE, early psum free)
                        if pending is not None:
                            emit_av(pending)
                        # exp (+ per-key row bias) -> bf16, both heads at once
                        pt2 = ptp.tile([128, 2, Q], BF16, tag="pt")
                        nc.scalar.activation(pt2, qk2, AF.Exp,
                                             bias=br_sb[:, kc:kc + 1], scale=1.0)
                        pts = [pt2[:, 0], pt2[:, 1]]
                        pending = (heads, pts, kc)
                emit_av(pending)
                pending_post = make_post(b, avt, smt, gate)
            pending_post()

    nc.compile()
    return nc


def make_in_maps(q_data, m_data, bias, nonbatched_bias, batched_bias,
                 query_w, key_w, value_w, gating_w, gating_b, output_w, output_b):
    """Host-side layout prep (transpose/reshape only) + sharding over 8 cores."""
    import ml_dtypes
    f = np.float32
    bf = ml_dtypes.bfloat16
    qT = np.ascontiguousarray(np.asarray(q_data, f).transpose(0, 2, 1).astype(bf))
    mT = np.ascontiguousarray(np.asarray(m_data, f).transpose(0, 2, 1).astype(bf))
    br = np.ascontiguousarray(np.asarray(bias, f).reshape(B, K))
    bbT = np.ascontiguousarray(
        np.asarray(batched_bias, f).transpose(0, 1, 3, 2).astype(bf))
    nbT = np.ascontiguousarray(
        np.asarray(nonbatched_bias, f).transpose(0, 2, 1).astype(bf))
    qw = np.ascontiguousarray(np.asarray(query_w, f).reshape(A, H * C))
    kw = np.ascontiguousarray(np.asarray(key_w, f).reshape(A, H * C))
    vw = np.ascontiguousarray(np.asarray(value_w, f).reshape(A, H * C))
    gw = np.ascontiguousarray(np.asarray(gating_w, f).reshape(A, H * C))
    gb = np.ascontiguousarray(np.asarray(gating_b, f).reshape(H * C))
    ow = np.ascontiguousarray(np.asarray(output_w, f).reshape(H * C, O))
    ob = np.ascontiguousarray(np.asarray(output_b, f))
    in_maps = []
    for c in range(CORES):
        s = slice(c * BLOC, (c + 1) * BLOC)
        in_maps.append({
            "qT": qT[s], "mT": mT[s], "biasr": br[s], "bbT": bbT[s], "nbT": nbT,
            "qw": qw, "kw": kw, "vw": vw, "gw": gw, "gb": gb, "ow": ow, "ob": ob,
        })
    return in_maps


_NC_CACHE = {}


def get_nc():
    if "nc" not in _NC_CACHE:
        _NC_CACHE["nc"] = build_nc()
    return _NC_CACHE["nc"]


def kernel(**inputs):
    in_maps = make_in_maps(**inputs)
    nc = get_nc()
    res = run_bass_kernel_spmd(nc, in_maps, core_ids=list(range(CORES)))
    outs = [res.results[c]["out"].reshape(BLOC, O, Q).transpose(0, 2, 1)
            for c in range(CORES)]
    return np.ascontiguousarray(np.concatenate(outs, axis=0))


# revision 23
# speedup vs baseline: 1.0583x; 1.0583x over previous
"""Gated multi-head attention (AlphaFold-style) on 8 TRN2 NeuronCores.

Sharding: data-parallel over batch B=32 -> 4 batches per core; zero collectives.

Layout strategy ("transposed land"): all on-device tensors keep the softmax
key axis (k) on SBUF partitions so the big bias tensors stream in naturally
after a host-side transpose, exp() fuses the per-key row bias via the ACT
bias port, and the softmax denominator comes out of the PE via a ones[128,32]
stationary matmul (which also pre-broadcasts 1/sum across each head's 32
partition rows for free). Host-side work is layout-only (transpose/reshape);
all arithmetic runs on device, bf16 matmuls with fp32 PSUM accumulation.

  qhT[hc, q]  = (query_w*scale)[a,hc]^T @ q_dataT[a,q]           (PE)
  khT[hc, k]  =  key_w^T @ m_dataT                               (PE)
  vb[k, hc]   =  (m_dataT^T-chunks @ value_w) -> bf16            (PE + DVE)
  gateT[hc,q] =  sigmoid(gating_w^T @ q_dataT + gating_b)        (PE + ACT)
  logitsT(h)[k,q] = khT_h^T-slices @ qhT_h  (row-tiled 4 heads)  (PE)
  psum += Id @ (nbT + bbT)                  (PE id-add; DVE pre-add)
  PT(h)[k,q]  = exp(psum + bias_row[k])  -> bf16                 (ACT)
  avT, sums   = col-tiled matmuls over k; AV lags one group
                behind exp in the PE stream to stay warm         (PE, bf16)
  wag         = avT * gateT * approx(1/sums)                     (DVE)
  outT[o, q]  = output_w^T-chunks @ wag + output_b               (PE + DVE)

Output is produced as [o, q] per batch and un-transposed on the host.
"""

import numpy as np

import concourse.bass as bass
import concourse.mybir as mybir
from concourse import bacc
from concourse.tile import TileContext
from concourse.masks import make_identity
from concourse.bass_utils import run_bass_kernel_spmd

B, Q, K, A, H, C, O = 32, 512, 512, 256, 8, 32, 256
CORES = 8
BLOC = B // CORES          # batches per core
NKC = K // 128             # k chunks
F32 = mybir.dt.float32
BF16 = mybir.dt.bfloat16
KEY_SCALE = float(C) ** -0.5
AF = mybir.ActivationFunctionType


def build_nc():
    nc = bacc.Bacc(None, target_bir_lowering=False)

    # --- DRAM parameters (per-core shards; names match in_maps keys) ---
    p_qT = nc.declare_dram_parameter("qT", [BLOC, A, Q], BF16, isOutput=False)
    p_mT = nc.declare_dram_parameter("mT", [BLOC, A, K], BF16, isOutput=False)
    p_br = nc.declare_dram_parameter("biasr", [BLOC, K], F32, isOutput=False)
    p_bbT = nc.declare_dram_parameter("bbT", [BLOC, H, K, Q], BF16, isOutput=False)
    p_nbT = nc.declare_dram_parameter("nbT", [H, K, Q], BF16, isOutput=False)
    p_qw = nc.declare_dram_parameter("qw", [A, H * C], F32, isOutput=False)
    p_kw = nc.declare_dram_parameter("kw", [A, H * C], F32, isOutput=False)
    p_vw = nc.declare_dram_parameter("vw", [A, H * C], F32, isOutput=False)
    p_gw = nc.declare_dram_parameter("gw", [A, H * C], F32, isOutput=False)
    p_gb = nc.declare_dram_parameter("gb", [H * C], F32, isOutput=False)
    p_ow = nc.declare_dram_parameter("ow", [H * C, O], F32, isOutput=False)
    p_ob = nc.declare_dram_parameter("ob", [O], F32, isOutput=False)
    p_out = nc.declare_dram_parameter("out", [BLOC, O, Q], F32, isOutput=True)

    with TileContext(nc) as tc:
        with (
            tc.tile_pool(name="const", bufs=1) as const,
            tc.tile_pool(name="nbres", bufs=1) as nbres,
            tc.tile_pool(name="data", bufs=2) as data,
            tc.tile_pool(name="proj", bufs=2) as proj,
            tc.tile_pool(name="bbt", bufs=12) as bbtp,
            tc.tile_pool(name="nbb", bufs=12) as nbbp,
            tc.tile_pool(name="pt", bufs=8) as ptp,
            tc.tile_pool(name="post", bufs=2) as post,
            tc.tile_pool(name="ps", bufs=3, space="PSUM") as psp,
            tc.tile_pool(name="avps", bufs=1, space="PSUM") as avps,
            tc.tile_pool(name="sumps", bufs=1, space="PSUM") as sumps,
        ):
            # ---------- one-time constants ----------
            ident = const.tile([128, 128], BF16)
            make_identity(nc, ident)
            ones = const.tile([128, 32], BF16)
            nc.vector.memset(ones, 1.0)

            # weights: [a, hc] -> [128, ka, hc], cast to bf16 on device;
            # key_scale folded into qw here.
            qw_sb = const.tile([128, 2, 256], BF16)
            kw_sb = const.tile([128, 2, 256], BF16)
            vw_sb = const.tile([128, 2, 256], BF16)
            gw_sb = const.tile([128, 2, 256], BF16)
            ow_sb = const.tile([128, 2, 256], BF16)
            for t, p, pat, scl in (
                (qw_sb, p_qw, "(ka p) hc -> p ka hc", KEY_SCALE),
                (kw_sb, p_kw, "(ka p) hc -> p ka hc", None),
                (vw_sb, p_vw, "(ka p) hc -> p ka hc", None),
                (gw_sb, p_gw, "(ka p) hc -> p ka hc", None),
                (ow_sb, p_ow, "(kh p) o -> p kh o", None),
            ):
                wstage = data.tile([128, 2, 256], F32, tag="stage")
                nc.sync.dma_start(out=wstage, in_=p.rearrange(pat, p=128))
                if scl is None:
                    nc.vector.tensor_copy(out=t, in_=wstage)
                else:
                    nc.vector.tensor_scalar_mul(out=t, in0=wstage, scalar1=scl)
            gb_sb = const.tile([128, 2], F32)
            nc.sync.dma_start(out=gb_sb, in_=p_gb.rearrange("(m p) -> p m", p=128))
            ob_sb = const.tile([128, 2], F32)
            nc.sync.dma_start(out=ob_sb, in_=p_ob.rearrange("(m p) -> p m", p=128))

            # nonbatched bias resident as bf16: [128, h, kc, q]
            # (loaded lazily, interleaved into batch 0's attention pipeline)
            nbt16 = nbres.tile([128, H, NKC, Q], BF16)

            # ---------- per-batch pipeline ----------
            # post(b-1) is emitted after proj(b) so the PE can run batch b's
            # projections while the DVE finishes b-1's normalize chain.
            def make_post(b, avt, smt, gate):
                def post_fn():
                    recb = post.tile([128, 2, Q], F32, tag="recb")
                    for t in range(2):
                        nc.vector.reciprocal_approx_fast(
                            out=recb[:, t], in_=smt[t])
                    grec = post.tile([128, 2, Q], F32, tag="grec")
                    wag = post.tile([128, 2, Q], BF16, tag="wag")
                    for t in range(2):
                        nc.vector.tensor_mul(
                            out=grec[:, t], in0=gate[:, t], in1=recb[:, t])
                        nc.vector.tensor_mul(
                            out=wag[:, t], in0=avt[t], in1=grec[:, t])
                    outT = post.tile([128, 2, Q], F32, tag="outT")
                    po2 = psp.tile([128, 2, Q], F32, tag="mm")
                    for mo in range(2):
                        oslc = slice(mo * 128, (mo + 1) * 128)
                        for kh in range(2):
                            nc.tensor.matmul(
                                po2[:, mo], ow_sb[:, kh, oslc], wag[:, kh],
                                start=(kh == 0), stop=(kh == 1))
                    for mo in range(2):
                        nc.vector.tensor_scalar_add(
                            out=outT[:, mo], in0=po2[:, mo],
                            scalar1=ob_sb[:, mo:mo + 1])
                    nc.gpsimd.dma_start(
                        out=p_out[b].rearrange("(mo p) q -> p mo q", p=128),
                        in_=outT)
                return post_fn

            pending_post = None
            for b in range(BLOC):
                qT_sb = data.tile([128, 2, Q], BF16, tag="qT")
                nc.sync.dma_start(
                    out=qT_sb, in_=p_qT[b].rearrange("(ka p) q -> p ka q", p=128)
                )
                mT_sb = data.tile([128, 2, K], BF16, tag="mT")
                nc.sync.dma_start(
                    out=mT_sb, in_=p_mT[b].rearrange("(ka p) q -> p ka q", p=128)
                )
                br_sb = data.tile([128, NKC], F32, tag="br")
                nc.sync.dma_start(
                    out=br_sb, in_=p_br[b].rearrange("(kc p) -> p kc", p=128)
                )

                # --- projections ---
                qhT = proj.tile([128, 2, Q], BF16, tag="qhT")
                khT = proj.tile([128, 2, K], BF16, tag="khT")
                gate = proj.tile([128, 2, Q], F32, tag="gate")
                for m in range(2):
                    mslc = slice(m * 128, (m + 1) * 128)
                    pqk = psp.tile([128, 2, Q], F32, tag="mm")
                    pgv = psp.tile([128, 2, Q], F32, tag="mm")
                    pq, pk, pg = pqk[:, 0], pqk[:, 1], pgv[:, 0]
                    for ka in range(2):
                        st, sp = ka == 0, ka == 1
                        nc.tensor.matmul(
                            pq, qw_sb[:, ka, mslc], qT_sb[:, ka], start=st, stop=sp)
                        nc.tensor.matmul(
                            pk, kw_sb[:, ka, mslc], mT_sb[:, ka], start=st, stop=sp)
                        nc.tensor.matmul(
                            pg, gw_sb[:, ka, mslc], qT_sb[:, ka], start=st, stop=sp)
                    nc.vector.tensor_copy(out=qhT[:, m], in_=pq)
                    nc.vector.tensor_copy(out=khT[:, m], in_=pk)
                    nc.scalar.activation(gate[:, m], pg, AF.Sigmoid,
                                         bias=gb_sb[:, m:m + 1], scale=1.0)

                # v projection -> bf16 [k(part), kc, hc]
                vb = proj.tile([128, NKC, 256], BF16, tag="vb")
                for kch in range(2):
                    pv2 = psp.tile([128, 2, Q], F32, tag="mm")
                    for kci in range(2):
                        kc = 2 * kch + kci
                        kslc = slice(kc * 128, (kc + 1) * 128)
                        pv = pv2[:, kci, 0:256]
                        for ka in range(2):
                            nc.tensor.matmul(
                                pv, mT_sb[:, ka, kslc], vw_sb[:, ka],
                                start=(ka == 0), stop=(ka == 1))
                        nc.vector.tensor_copy(out=vb[:, kc], in_=pv)

                # previous batch's normalize/output tail
                if pending_post is not None:
                    pending_post()
                    pending_post = None

                # --- attention core ---
                av0 = avps.tile([128, Q], F32, tag="av")     # heads 0-3
                av1 = avps.tile([128, Q], F32, tag="av")     # heads 4-7
                sm0 = sumps.tile([128, Q], F32, tag="sm")    # per-head sums x32
                sm1 = sumps.tile([128, Q], F32, tag="sm")
                avt = (av0, av1)
                smt = (sm0, sm1)

                def emit_av(g):
                    g_heads, g_pts, g_kc = g
                    for i2, h2 in enumerate(g_heads):
                        j2 = h2 % 4
                        nc.tensor.matmul(
                            avt[h2 // 4][32 * j2:32 * j2 + 32],
                            vb[:, g_kc, 32 * h2:32 * h2 + 32],
                            g_pts[i2],
                            start=(g_kc == 0), stop=(g_kc == NKC - 1),
                            tile_position=(0, 32 * j2), skip_group_check=True)
                    for i2, h2 in enumerate(g_heads):
                        j2 = h2 % 4
                        nc.tensor.matmul(
                            smt[h2 // 4][32 * j2:32 * j2 + 32],
                            ones, g_pts[i2],
                            start=(g_kc == 0), stop=(g_kc == NKC - 1),
                            tile_position=(0, 32 * j2), skip_group_check=True)

                pending = None
                for kc in range(NKC):
                    kslc = slice(kc * 128, (kc + 1) * 128)
                    for sg in range(4):       # subgroup: heads 2*sg, 2*sg+1
                        hs = sg // 2
                        heads = [2 * sg, 2 * sg + 1]
                        if b == 0 and kc == 0:
                            for h in heads:
                                nc.sync.dma_start(
                                    out=nbt16[:, h],
                                    in_=p_nbT[h].rearrange(
                                        "(kc2 p) q -> p kc2 q", p=128),
                                )
                        # pre-add biases on DVE (one op per head)
                        nbbs = []
                        for i, h in enumerate(heads):
                            bbt = bbtp.tile([128, Q], BF16, tag="bbt")
                            nc.sync.dma_start(out=bbt, in_=p_bbT[b, h, kslc])
                            nbb = nbbp.tile([128, Q], BF16, tag="nbb")
                            nc.vector.tensor_add(
                                out=nbb, in0=nbt16[:, h, kc], in1=bbt)
                            nbbs.append(nbb)
                        # row-tiled QK^T (2 heads concurrent, one 2-bank tile)
                        qk2 = psp.tile([128, 2, Q], F32, tag="mm")
                        for i, h in enumerate(heads):
                            j = h % 4
                            jslc = slice(32 * j, 32 * j + 32)
                            nc.tensor.matmul(
                                qk2[:, i],
                                khT[jslc, h // 4, kslc],
                                qhT[jslc, h // 4],
                                start=True, stop=False,
                                tile_position=(32 * j, 0))
                        # identity-add of biases into psum
                        for i, h in enumerate(heads):
                            nc.tensor.matmul(
                                qk2[:, i], ident, nbbs[i],
                                start=False, stop=True)
                        # AV/sums of the PREVIOUS subgroup fill the PE while
                        # this one's exp runs (warm PE, early psum free)
                        if pending is not None:
                            emit_av(pending)
                        # exp (+ per-key row bias) -> bf16, both heads at once
                        pt2 = ptp.tile([128, 2, Q], BF16, tag="pt")
                        nc.scalar.activation(pt2, qk2, AF.Exp,
                                             bias=br_sb[:, kc:kc + 1], scale=1.0)
                        pts = [pt2[:, 0], pt2[:, 1]]
                        pending = (heads, pts, kc)
                emit_av(pending)
                pending_post = make_post(b, avt, smt, gate)
            pending_post()

    nc.compile()
    return nc


def make_in_maps(q_data, m_data, bias, nonbatched_bias, batched_bias,
                 query_w, key_w, value_w, gating_w, gating_b, output_w, output_b):
    """Host-side layout prep (transpose/reshape only) + sharding over 8 cores."""
    import ml_dtypes
    f = np.float32
    bf = ml_dtypes.bfloat16
    qT = np.ascontiguousarray(np.asarray(q_data, f).transpose(0, 2, 1).astype(bf))
    mT = np.ascontiguousarray(np.asarray(m_data, f).transpose(0, 2, 1).astype(bf))
    br = np.ascontiguousarray(np.asarray(bias, f).reshape(B, K))
    bbT = np.ascontiguousarray(
        np.asarray(batched_bias, f).transpose(0, 1, 3, 2).astype(bf))
    nbT = np.ascontiguousarray(
        np.asarray(nonbatched_bias, f).transpose(0, 2, 1).astype(bf))
    qw = np.ascontiguousarray(np.asarray(query_w, f).reshape(A, H * C))
    kw = np.ascontiguousarray(np.asarray(key_w, f).reshape(A, H * C))
    vw = np.ascontiguousarray(np.asarray(value_w, f).reshape(A, H * C))
    gw = np.ascontiguousarray(np.asarray(gating_w, f).reshape(A, H * C))
    gb = np.ascontiguousarray(np.asarray(gating_b, f).reshape(H * C))
    ow = np.ascontiguousarray(np.asarray(output_w, f).reshape(H * C, O))
    ob = np.ascontiguousarray(np.asarray(output_b, f))
    in_maps = []
    for c in range(CORES):
        s = slice(c * BLOC, (c + 1) * BLOC)
        in_maps.append({
            "qT": qT[s], "mT": mT[s], "biasr": br[s], "bbT": bbT[s], "nbT": nbT,
            "qw": qw, "kw": kw, "vw": vw, "gw": gw, "gb": gb, "ow": ow, "ob": ob,
        })
    return in_maps


_NC_CACHE = {}


def get_nc():
    if "nc" not in _NC_CACHE:
        _NC_CACHE["nc"] = build_nc()
    return _NC_CACHE["nc"]


def kernel(**inputs):
    in_maps = make_in_maps(**inputs)
    nc = get_nc()
    res = run_bass_kernel_spmd(nc, in_maps, core_ids=list(range(CORES)))
    outs = [res.results[c]["out"].reshape(BLOC, O, Q).transpose(0, 2, 1)
            for c in range(CORES)]
    return np.ascontiguousarray(np.concatenate(outs, axis=0))


# revision 25
# speedup vs baseline: 1.2038x; 1.1375x over previous
"""Gated multi-head attention (AlphaFold-style) on 8 TRN2 NeuronCores.

Sharding: data-parallel over batch B=32 -> 4 batches per core; zero collectives.

Layout strategy ("transposed land"): all on-device tensors keep the softmax
key axis (k) on SBUF partitions so the big bias tensors stream in naturally
after a host-side transpose, exp() fuses the per-key row bias via the ACT
bias port, and the softmax denominator comes out of the PE via a ones[128,32]
stationary matmul (which also pre-broadcasts 1/sum across each head's 32
partition rows for free). Host-side work is layout-only (transpose/reshape);
all arithmetic runs on device, bf16 matmuls with fp32 PSUM accumulation.

  qhT[hc, q]  = (query_w*scale)[a,hc]^T @ q_dataT[a,q]           (PE)
  khT[hc, k]  =  key_w^T @ m_dataT                               (PE)
  vb[k, hc]   =  (m_dataT^T-chunks @ value_w) -> bf16            (PE + DVE)
  gateT[hc,q] =  sigmoid(gating_w^T @ q_dataT + gating_b)        (PE + ACT)
  logitsT(h)[k,q] = khT_h^T-slices @ qhT_h  (row-tiled 4 heads)  (PE)
  psum += Id @ (nbT + bbT)                  (PE id-add; DVE pre-add)
  PT(h)[k,q]  = exp(psum + bias_row[k])  -> bf16                 (ACT)
  avT, sums   = col-tiled matmuls over k; AV lags one group
                behind exp in the PE stream to stay warm         (PE, bf16)
  wag         = avT * gateT * approx(1/sums)                     (DVE)
  outT[o, q]  = output_w^T-chunks @ wag + output_b               (PE + DVE)

Output is produced as [o, q] per batch and un-transposed on the host.
"""

import numpy as np

import concourse.bass as bass
import concourse.mybir as mybir
from concourse import bacc
from concourse.tile import TileContext
from concourse.masks import make_identity
from concourse.bass_utils import run_bass_kernel_spmd

B, Q, K, A, H, C, O = 32, 512, 512, 256, 8, 32, 256
CORES = 8
BLOC = B // CORES          # batches per core
NKC = K // 128             # k chunks
F32 = mybir.dt.float32
BF16 = mybir.dt.bfloat16
KEY_SCALE = float(C) ** -0.5
AF = mybir.ActivationFunctionType


def build_nc():
    nc = bacc.Bacc(None, target_bir_lowering=False)

    # --- DRAM parameters (per-core shards; names match in_maps keys) ---
    p_qT = nc.declare_dram_parameter("qT", [BLOC, A, Q], BF16, isOutput=False)
    p_mT = nc.declare_dram_parameter("mT", [BLOC, A, K], BF16, isOutput=False)
    p_br = nc.declare_dram_parameter("biasr", [BLOC, K], F32, isOutput=False)
    p_bbT = nc.declare_dram_parameter("bbT", [BLOC, H, K, Q], BF16, isOutput=False)
    p_nbT = nc.declare_dram_parameter("nbT", [H, K, Q], BF16, isOutput=False)
    p_qw = nc.declare_dram_parameter("qw", [A, H * C], F32, isOutput=False)
    p_kw = nc.declare_dram_parameter("kw", [A, H * C], F32, isOutput=False)
    p_vw = nc.declare_dram_parameter("vw", [A, H * C], F32, isOutput=False)
    p_gw = nc.declare_dram_parameter("gw", [A, H * C], F32, isOutput=False)
    p_gb = nc.declare_dram_parameter("gb", [H * C], F32, isOutput=False)
    p_ow = nc.declare_dram_parameter("ow", [H * C, O], F32, isOutput=False)
    p_ob = nc.declare_dram_parameter("ob", [O], F32, isOutput=False)
    p_out = nc.declare_dram_parameter("out", [BLOC, O, Q], F32, isOutput=True)

    with TileContext(nc) as tc:
        with (
            tc.tile_pool(name="const", bufs=1) as const,
            tc.tile_pool(name="nbres", bufs=1) as nbres,
            tc.tile_pool(name="data", bufs=4) as data,
            tc.tile_pool(name="proj", bufs=4) as proj,
            tc.tile_pool(name="bbt", bufs=12) as bbtp,
            tc.tile_pool(name="nbb", bufs=12) as nbbp,
            tc.tile_pool(name="pt", bufs=8) as ptp,
            tc.tile_pool(name="post", bufs=2) as post,
            tc.tile_pool(name="ps", bufs=3, space="PSUM") as psp,
            tc.tile_pool(name="avps", bufs=1, space="PSUM") as avps,
            tc.tile_pool(name="sumps", bufs=1, space="PSUM") as sumps,
        ):
            # ---------- one-time constants ----------
            ident = const.tile([128, 128], BF16)
            make_identity(nc, ident)
            ones = const.tile([128, 32], BF16)
            nc.vector.memset(ones, 1.0)

            # weights: [a, hc] -> [128, ka, hc], cast to bf16 on device;
            # key_scale folded into qw here.
            qw_sb = const.tile([128, 2, 256], BF16)
            kw_sb = const.tile([128, 2, 256], BF16)
            vw_sb = const.tile([128, 2, 256], BF16)
            gw_sb = const.tile([128, 2, 256], BF16)
            ow_sb = const.tile([128, 2, 256], BF16)
            for t, p, pat, scl in (
                (qw_sb, p_qw, "(ka p) hc -> p ka hc", KEY_SCALE),
                (kw_sb, p_kw, "(ka p) hc -> p ka hc", None),
                (vw_sb, p_vw, "(ka p) hc -> p ka hc", None),
                (gw_sb, p_gw, "(ka p) hc -> p ka hc", None),
                (ow_sb, p_ow, "(kh p) o -> p kh o", None),
            ):
                wstage = data.tile([128, 2, 256], F32, tag="stage")
                nc.sync.dma_start(out=wstage, in_=p.rearrange(pat, p=128))
                if scl is None:
                    nc.vector.tensor_copy(out=t, in_=wstage)
                else:
                    nc.vector.tensor_scalar_mul(out=t, in0=wstage, scalar1=scl)
            gb_sb = const.tile([128, 2], F32)
            nc.sync.dma_start(out=gb_sb, in_=p_gb.rearrange("(m p) -> p m", p=128))
            ob_sb = const.tile([128, 2], F32)
            nc.sync.dma_start(out=ob_sb, in_=p_ob.rearrange("(m p) -> p m", p=128))

            # nonbatched bias resident as bf16: [128, h, kc, q]
            # (loaded lazily, interleaved into batch 0's attention pipeline)
            nbt16 = nbres.tile([128, H, NKC, Q], BF16)

            # ---------- per-batch pipeline ----------
            # post(b-1) is emitted after proj(b) so the PE can run batch b's
            # projections while the DVE finishes b-1's normalize chain.
            def make_post(b, avt, smt, gate):
                def post_fn():
                    recb = post.tile([128, 2, Q], F32, tag="recb")
                    for t in range(2):
                        nc.vector.reciprocal_approx_fast(
                            out=recb[:, t], in_=smt[t])
                    grec = post.tile([128, 2, Q], F32, tag="grec")
                    wag = post.tile([128, 2, Q], BF16, tag="wag")
                    for t in range(2):
                        nc.vector.tensor_mul(
                            out=grec[:, t], in0=gate[:, t], in1=recb[:, t])
                        nc.vector.tensor_mul(
                            out=wag[:, t], in0=avt[t], in1=grec[:, t])
                    outT = post.tile([128, 2, Q], F32, tag="outT")
                    po2 = psp.tile([128, 2, Q], F32, tag="mm")
                    for mo in range(2):
                        oslc = slice(mo * 128, (mo + 1) * 128)
                        for kh in range(2):
                            nc.tensor.matmul(
                                po2[:, mo], ow_sb[:, kh, oslc], wag[:, kh],
                                start=(kh == 0), stop=(kh == 1))
                    for mo in range(2):
                        nc.vector.tensor_scalar_add(
                            out=outT[:, mo], in0=po2[:, mo],
                            scalar1=ob_sb[:, mo:mo + 1])
                    nc.gpsimd.dma_start(
                        out=p_out[b].rearrange("(mo p) q -> p mo q", p=128),
                        in_=outT)
                return post_fn

            # ---------- hoisted loads + projections for ALL batches ----------
            # One projection phase up front: a single Sigmoid table residency,
            # then the attention phases run pure Exp with dense PE streams.
            br_l, qhT_l, khT_l, gate_l, vb_l = [], [], [], [], []
            qT_l, mT_l = [], []
            for b in range(BLOC):
                qT_sb = data.tile([128, 2, Q], BF16, tag="qT")
                nc.sync.dma_start(
                    out=qT_sb, in_=p_qT[b].rearrange("(ka p) q -> p ka q", p=128)
                )
                mT_sb = data.tile([128, 2, K], BF16, tag="mT")
                nc.sync.dma_start(
                    out=mT_sb, in_=p_mT[b].rearrange("(ka p) q -> p ka q", p=128)
                )
                br_sb = data.tile([128, NKC], F32, tag="br")
                nc.sync.dma_start(
                    out=br_sb, in_=p_br[b].rearrange("(kc p) -> p kc", p=128)
                )
                qT_l.append(qT_sb)
                mT_l.append(mT_sb)
                br_l.append(br_sb)

            for b in range(BLOC):
                qT_sb, mT_sb = qT_l[b], mT_l[b]
                qhT = proj.tile([128, 2, Q], BF16, tag="qhT")
                khT = proj.tile([128, 2, K], BF16, tag="khT")
                gate = proj.tile([128, 2, Q], F32, tag="gate")
                for m in range(2):
                    mslc = slice(m * 128, (m + 1) * 128)
                    pqk = psp.tile([128, 2, Q], F32, tag="mm")
                    pgv = psp.tile([128, 2, Q], F32, tag="mm")
                    pq, pk, pg = pqk[:, 0], pqk[:, 1], pgv[:, 0]
                    for ka in range(2):
                        st, sp = ka == 0, ka == 1
                        nc.tensor.matmul(
                            pq, qw_sb[:, ka, mslc], qT_sb[:, ka], start=st, stop=sp)
                        nc.tensor.matmul(
                            pk, kw_sb[:, ka, mslc], mT_sb[:, ka], start=st, stop=sp)
                        nc.tensor.matmul(
                            pg, gw_sb[:, ka, mslc], qT_sb[:, ka], start=st, stop=sp)
                    nc.vector.tensor_copy(out=qhT[:, m], in_=pq)
                    nc.vector.tensor_copy(out=khT[:, m], in_=pk)
                    nc.scalar.activation(gate[:, m], pg, AF.Sigmoid,
                                         bias=gb_sb[:, m:m + 1], scale=1.0)

                vb = proj.tile([128, NKC, 256], BF16, tag="vb")
                for kch in range(2):
                    pv2 = psp.tile([128, 2, Q], F32, tag="mm")
                    for kci in range(2):
                        kc = 2 * kch + kci
                        kslc = slice(kc * 128, (kc + 1) * 128)
                        pv = pv2[:, kci, 0:256]
                        for ka in range(2):
                            nc.tensor.matmul(
                                pv, mT_sb[:, ka, kslc], vw_sb[:, ka],
                                start=(ka == 0), stop=(ka == 1))
                        nc.vector.tensor_copy(out=vb[:, kc], in_=pv)
                qhT_l.append(qhT)
                khT_l.append(khT)
                gate_l.append(gate)
                vb_l.append(vb)

            pending_post = None
            for b in range(BLOC):
                qhT, khT, gate, vb = qhT_l[b], khT_l[b], gate_l[b], vb_l[b]
                br_sb = br_l[b]

                # --- attention core ---
                av0 = avps.tile([128, Q], F32, tag="av")     # heads 0-3
                av1 = avps.tile([128, Q], F32, tag="av")     # heads 4-7
                sm0 = sumps.tile([128, Q], F32, tag="sm")    # per-head sums x32
                sm1 = sumps.tile([128, Q], F32, tag="sm")
                avt = (av0, av1)
                smt = (sm0, sm1)

                def emit_av(g):
                    g_heads, g_pts, g_kc = g
                    for i2, h2 in enumerate(g_heads):
                        j2 = h2 % 4
                        nc.tensor.matmul(
                            avt[h2 // 4][32 * j2:32 * j2 + 32],
                            vb[:, g_kc, 32 * h2:32 * h2 + 32],
                            g_pts[i2],
                            start=(g_kc == 0), stop=(g_kc == NKC - 1),
                            tile_position=(0, 32 * j2), skip_group_check=True)
                    for i2, h2 in enumerate(g_heads):
                        j2 = h2 % 4
                        nc.tensor.matmul(
                            smt[h2 // 4][32 * j2:32 * j2 + 32],
                            ones, g_pts[i2],
                            start=(g_kc == 0), stop=(g_kc == NKC - 1),
                            tile_position=(0, 32 * j2), skip_group_check=True)

                pending = None
                for kc in range(NKC):
                    kslc = slice(kc * 128, (kc + 1) * 128)
                    for sg in range(4):       # subgroup: heads 2*sg, 2*sg+1
                        hs = sg // 2
                        heads = [2 * sg, 2 * sg + 1]
                        # previous batch's normalize tail, woven in after the
                        # first subgroup so the PE never waits on the DVE chain
                        if pending_post is not None and kc * 4 + sg == 1:
                            pending_post()
                            pending_post = None
                        if b == 0 and kc == 0:
                            for h in heads:
                                nc.sync.dma_start(
                                    out=nbt16[:, h],
                                    in_=p_nbT[h].rearrange(
                                        "(kc2 p) q -> p kc2 q", p=128),
                                )
                        # pre-add biases on DVE (one op per head)
                        nbbs = []
                        for i, h in enumerate(heads):
                            bbt = bbtp.tile([128, Q], BF16, tag="bbt")
                            nc.sync.dma_start(out=bbt, in_=p_bbT[b, h, kslc])
                            nbb = nbbp.tile([128, Q], BF16, tag="nbb")
                            nc.vector.tensor_add(
                                out=nbb, in0=nbt16[:, h, kc], in1=bbt)
                            nbbs.append(nbb)
                        # row-tiled QK^T (2 heads concurrent, one 2-bank tile)
                        qk2 = psp.tile([128, 2, Q], F32, tag="mm")
                        for i, h in enumerate(heads):
                            j = h % 4
                            jslc = slice(32 * j, 32 * j + 32)
                            nc.tensor.matmul(
                                qk2[:, i],
                                khT[jslc, h // 4, kslc],
                                qhT[jslc, h // 4],
                                start=True, stop=False,
                                tile_position=(32 * j, 0))
                        # identity-add of biases into psum
                        for i, h in enumerate(heads):
                            nc.tensor.matmul(
                                qk2[:, i], ident, nbbs[i],
                                start=False, stop=True)
                        # AV/sums of the PREVIOUS subgroup fill the PE while
                        # this one's exp runs (warm PE, early psum free)
                        if pending is not None:
                            emit_av(pending)
                        # exp (+ per-key row bias) -> bf16, both heads at once
                        pt2 = ptp.tile([128, 2, Q], BF16, tag="pt")
                        nc.scalar.activation(pt2, qk2, AF.Exp,
                                             bias=br_sb[:, kc:kc + 1], scale=1.0)
                        pts = [pt2[:, 0], pt2[:, 1]]
                        pending = (heads, pts, kc)
                emit_av(pending)
                pending_post = make_post(b, avt, smt, gate)
            pending_post()

    nc.compile()
    return nc


def make_in_maps(q_data, m_data, bias, nonbatched_bias, batched_bias,
                 query_w, key_w, value_w, gating_w, gating_b, output_w, output_b):
    """Host-side layout prep (transpose/reshape only) + sharding over 8 cores."""
    import ml_dtypes
    f = np.float32
    bf = ml_dtypes.bfloat16
    qT = np.ascontiguousarray(np.asarray(q_data, f).transpose(0, 2, 1).astype(bf))
    mT = np.ascontiguousarray(np.asarray(m_data, f).transpose(0, 2, 1).astype(bf))
    br = np.ascontiguousarray(np.asarray(bias, f).reshape(B, K))
    bbT = np.ascontiguousarray(
        np.asarray(batched_bias, f).transpose(0, 1, 3, 2).astype(bf))
    nbT = np.ascontiguousarray(
        np.asarray(nonbatched_bias, f).transpose(0, 2, 1).astype(bf))
    qw = np.ascontiguousarray(np.asarray(query_w, f).reshape(A, H * C))
    kw = np.ascontiguousarray(np.asarray(key_w, f).reshape(A, H * C))
    vw = np.ascontiguousarray(np.asarray(value_w, f).reshape(A, H * C))
    gw = np.ascontiguousarray(np.asarray(gating_w, f).reshape(A, H * C))
    gb = np.ascontiguousarray(np.asarray(gating_b, f).reshape(H * C))
    ow = np.ascontiguousarray(np.asarray(output_w, f).reshape(H * C, O))
    ob = np.ascontiguousarray(np.asarray(output_b, f))
    in_maps = []
    for c in range(CORES):
        s = slice(c * BLOC, (c + 1) * BLOC)
        in_maps.append({
            "qT": qT[s], "mT": mT[s], "biasr": br[s], "bbT": bbT[s], "nbT": nbT,
            "qw": qw, "kw": kw, "vw": vw, "gw": gw, "gb": gb, "ow": ow, "ob": ob,
        })
    return in_maps


_NC_CACHE = {}


def get_nc():
    if "nc" not in _NC_CACHE:
        _NC_CACHE["nc"] = build_nc()
    return _NC_CACHE["nc"]


def kernel(**inputs):
    in_maps = make_in_maps(**inputs)
    nc = get_nc()
    res = run_bass_kernel_spmd(nc, in_maps, core_ids=list(range(CORES)))
    outs = [res.results[c]["out"].reshape(BLOC, O, Q).transpose(0, 2, 1)
            for c in range(CORES)]
    return np.ascontiguousarray(np.concatenate(outs, axis=0))
